# revision 19
# baseline (speedup 1.0000x reference)
"""Half-Chamfer distance kernel for Trainium2 (8 NeuronCores).

Problem: prediction [4, 8192, 3], ground_truth [4, 8192, 3] (f32).
out[b] = mean_n min_m ||pred[b,n] - gt[b,m]||^2

Sharding: core c -> (batch b = c//2, N-half h = c%2). Each core computes
min over all M=8192 gt points for its 4096 prediction points, row-sums;
host combines the per-core [128] partial sums.

Device algorithm (per core), engine-balanced for this HW where the PE
streams moving data at ~1.2 GHz regardless of dtype and PSUM can only be
drained by VectorE (1 elem/cycle via its single PSUM port) and ScalarE
(copy at ~1.09 ns/elem):

  d2[n,m] exactly from fp16-quantized points via K=7 fp16 matmuls:
    stationary rows [x0, x1, x2, 1, 1, x2h, x2l]
    moving rows     [-2y0, -2y1, -2y2, qh, ql, 1, 1]
  (x2h+x2l = |x^|^2, qh+ql = |y^|^2 hi/lo fp16 splits of the f64 norms of
  the QUANTIZED points, so PSUM = |x^-y^|^2 to ~1e-5. Keeping d2 >= 0 in
  the matmul matters: small mins stay accurate in bf16 downstream.)

  Per n-tile (128 preds), M=8192 arrives as 8 PSUM chunks [128,1024]
  (2 matmuls each). Drain split tuned to measured rates:
    - chunks 0,4: consumed directly by VectorE TT-min against a copied
      chunk (1x, 1224ns) -> bf16
    - chunks 1,2,3,5,6,7: ScalarE-copied PSUM->SBUF as bf16 (1114ns)
    - bf16 merge tree on VectorE at 2x (692ns per [128,1024] TT-min)
    - final tensor_reduce min [128,1024] -> dx column (1219ns)
  DVE ~7.1us/n-tile, ACT ~6.7us, PE ~6.8us single-stream -> PE is row-
  tiled 2x (tile_position (0,0)/(32,0), inputs replicated at partition
  offset 32) so two n-tiles' matmuls stream concurrently (~3.4us each).

Tail: relu-clamp + row-sum on device; host sums 128 partials per core.
"""

import numpy as np

import concourse.bass as bass
import concourse.mybir as mybir
from concourse.bass_utils import run_bass_kernel_spmd
from concourse.tile import TileContext

B = 4
N = 8192
M = 8192
D = 3
N_CORES = 8
N_SH = N // 2          # 4096 prediction points per core
KR = 7                 # contraction rows
JC = 512               # cols per matmul (1 PSUM bank of fp32)
CP = 1024              # chunk width (2 matmuls per chunk)
NTILES = N_SH // 128   # 32 n-tiles of 128 partitions
CHUNKS = M // CP       # 8 chunks per n-tile
PSUM_DIRECT = (0, 4)   # chunks drained by DVE straight from PSUM
COPIED = (1, 2, 3, 5, 6, 7)

F32 = mybir.dt.float32
F16 = mybir.dt.float16
BF16 = mybir.dt.bfloat16

_CACHED_NC = None


def _build_nc():
    nc = bass.Bass()
    statx_d = nc.declare_dram_parameter("statx", [KR, N_SH], F16, isOutput=False)
    mov_d = nc.declare_dram_parameter("mov", [KR, M], F16, isOutput=False)
    out_d = nc.declare_dram_parameter("out", [128, 1], F32, isOutput=True)

    with TileContext(nc) as tc:
        with (
            tc.tile_pool(name="const", bufs=1) as cpool,
            tc.tile_pool(name="cp1", bufs=7) as copool,
            tc.tile_pool(name="cp2", bufs=3) as co2pool,
            tc.tile_pool(name="tr", bufs=14) as trpool,
            tc.tile_pool(name="ps1", bufs=2, space="PSUM") as ps1pool,
            tc.tile_pool(name="ps2", bufs=1, space="PSUM") as ps2pool,
        ):
            # inputs replicated at partition offsets 0 and 32 (PE row bands)
            statx = cpool.tile([39, N_SH], F16, tag="statx")
            mov = cpool.tile([39, M], F16, tag="mov")
            dx_all = cpool.tile([128, NTILES], F32, tag="dx")
            nc.sync.dma_start(out=statx[0:KR, :], in_=statx_d[:])
            nc.sync.dma_start(out=statx[32:32 + KR, :], in_=statx_d[:])
            nc.sync.dma_start(out=mov[0:KR, :], in_=mov_d[:])
            nc.sync.dma_start(out=mov[32:32 + KR, :], in_=mov_d[:])

            def tt_min(dst, a, b):
                nc.vector.tensor_tensor(
                    out=dst, in0=a, in1=b, op=mybir.AluOpType.min
                )

            def emit_tree(pend):
                """DVE merge tree for a completed tile (software-pipelined:
                runs one iteration later so fresh PSUM TTs lead the queue)."""
                a0, a1, s23, s6, s7, tp = pend
                b0 = trpool.tile([128, CP], BF16, tag="a")
                tt_min(b0[:], s23[:, :CP], s23[:, CP:])
                b1 = trpool.tile([128, CP], BF16, tag="a")
                tt_min(b1[:], s6[:], s7[:])
                c0 = trpool.tile([128, CP], BF16, tag="a")
                tt_min(c0[:], a0[:], a1[:])
                c1 = trpool.tile([128, CP], BF16, tag="a")
                tt_min(c1[:], b0[:], b1[:])
                dfin = trpool.tile([128, CP], BF16, tag="a")
                tt_min(dfin[:], c0[:], c1[:])
                nc.vector.tensor_reduce(
                    out=dx_all[:, tp:tp + 1], in_=dfin[:],
                    axis=mybir.AxisListType.X, op=mybir.AluOpType.min,
                )

            pend = None
            for t in range(NTILES):
                base = 32 * (t % 2)     # PE row band for this n-tile
                lhs = statx[base:base + KR, t * 128:(t + 1) * 128]

                def mm_into(p, off, k2):
                    """two matmuls covering moving cols [k2*1024, +1024)"""
                    for j in range(2):
                        sl = slice(k2 * CP + j * JC, k2 * CP + (j + 1) * JC)
                        nc.tensor.matmul(
                            out=p[:, off + j * JC:off + (j + 1) * JC],
                            lhsT=lhs, rhs=mov[base:base + KR, sl],
                            start=True, stop=True,
                            tile_position=(base, 0),
                        )

                # chunks 0,1: a0 = TT(psum0, bf16 copy of 1)
                p0 = ps1pool.tile([128, CP], F32, tag="ps")
                mm_into(p0, 0, 0)
                p1 = ps1pool.tile([128, CP], F32, tag="ps")
                mm_into(p1, 0, 1)
                s1 = copool.tile([128, CP], BF16, tag="s")
                nc.scalar.copy(out=s1[:], in_=p1[:])
                a0 = trpool.tile([128, CP], BF16, tag="a")
                tt_min(a0[:], p0[:], s1[:])

                # chunks 2,3: one wide PSUM tile, one wide ScalarE copy
                p23 = ps2pool.tile([128, 2 * CP], F32, tag="p23")
                mm_into(p23, 0, 2)
                mm_into(p23, CP, 3)
                s23 = co2pool.tile([128, 2 * CP], BF16, tag="s23")
                nc.scalar.copy(out=s23[:], in_=p23[:])

                # previous tile's merge tree fills the DVE queue here
                if pend is not None:
                    emit_tree(pend)

                # chunks 4,5: a1 = TT(psum4, bf16 copy of 5)
                p4 = ps1pool.tile([128, CP], F32, tag="ps")
                mm_into(p4, 0, 4)
                p5 = ps1pool.tile([128, CP], F32, tag="ps")
                mm_into(p5, 0, 5)
                s5 = copool.tile([128, CP], BF16, tag="s")
                nc.scalar.copy(out=s5[:], in_=p5[:])
                a1 = trpool.tile([128, CP], BF16, tag="a")
                tt_min(a1[:], p4[:], s5[:])

                # chunks 6,7: copied
                p6 = ps1pool.tile([128, CP], F32, tag="ps")
                mm_into(p6, 0, 6)
                s6 = copool.tile([128, CP], BF16, tag="s")
                nc.scalar.copy(out=s6[:], in_=p6[:])
                p7 = ps1pool.tile([128, CP], F32, tag="ps")
                mm_into(p7, 0, 7)
                s7 = copool.tile([128, CP], BF16, tag="s")
                nc.scalar.copy(out=s7[:], in_=p7[:])

                pend = (a0, a1, s23, s6, s7, t)
            emit_tree(pend)

            # clamp at 0 (matches reference's maximum(d2, 0) before min)
            nc.vector.tensor_scalar_max(
                out=dx_all[:], in0=dx_all[:], scalar1=0.0
            )
            dxsum = cpool.tile([128, 1], F32, tag="dxsum")
            nc.vector.tensor_reduce(
                out=dxsum[:], in_=dx_all[:],
                axis=mybir.AxisListType.X, op=mybir.AluOpType.add,
            )
            nc.sync.dma_start(out=out_d[:], in_=dxsum[:])

    # Populate .instr bytes for InstISA subclasses; this walrus errors
    # "ISA wrong length" on empty payloads.
    mybir.codegen_inst_isa_subclasses(nc)
    _legalize_for_walrus(nc)
    return nc


def _legalize_for_walrus(nc, max_waits=1):
    """This container's walrus encodes at most one sync-wait per
    instruction (fused-LW matmuls, drains, ...) and cannot encode
    EVENT_SEMAPHORE_RANGE_CLEAR at all.  Spill extra waits onto
    standalone NoOps queued just before on the same engine, and drop the
    tail sem range-clear."""
    RANGE_CLEAR_OPCODE = 176
    for f in nc.m.functions:
        for blk in f.blocks:
            out = []
            for inst in blk.instructions:
                if (
                    type(inst).__name__ == "InstISA"
                    and getattr(inst, "isa_opcode", None) == RANGE_CLEAR_OPCODE
                ):
                    continue
                si = inst.sync_info
                if si is not None and len(si.on_wait) > max_waits:
                    waits = list(si.on_wait)
                    for w in waits[:-max_waits]:
                        out.append(mybir.InstNoOp(
                            name=nc.get_next_instruction_name(),
                            engine=inst.engine,
                            sync_info=mybir.SyncInfo(
                                on_wait=[w], on_update=[]),
                        ))
                    inst.sync_info = mybir.SyncInfo(
                        on_wait=waits[-max_waits:],
                        on_update=list(si.on_update),
                    )
                out.append(inst)
            blk.instructions = out


def _get_nc():
    global _CACHED_NC
    if _CACHED_NC is None:
        _CACHED_NC = _build_nc()
    return _CACHED_NC


def _prep_core_inputs(x, y):
    """x: [N_SH, 3] f32 pred slice; y: [M, 3] f32 gt batch.

    Quantize points to fp16; compute the squared norms of the QUANTIZED
    points in f64 and hi/lo-split them into fp16 pairs, so the matmul's
    fp32 accumulation reconstructs |x^ - y^|^2 to ~1e-5 absolute."""
    xq = x.astype(np.float16)
    yq = y.astype(np.float16)
    x64 = xq.astype(np.float64)
    y64 = yq.astype(np.float64)

    x2 = (x64 * x64).sum(-1)
    x2h = x2.astype(np.float16)
    x2l = (x2 - x2h.astype(np.float64)).astype(np.float16)

    q = (y64 * y64).sum(-1)
    qh = q.astype(np.float16)
    ql = (q - qh.astype(np.float64)).astype(np.float16)

    ones_n = np.ones(N_SH, np.float16)
    ones_m = np.ones(M, np.float16)
    m2 = (-2.0 * y64).astype(np.float16)  # exact: -2 * fp16 value

    statx = np.stack([xq[:, 0], xq[:, 1], xq[:, 2], ones_n, ones_n, x2h, x2l])
    mov = np.stack([m2[:, 0], m2[:, 1], m2[:, 2], qh, ql, ones_m, ones_m])
    return {
        "statx": np.ascontiguousarray(statx, dtype=np.float16),
        "mov": np.ascontiguousarray(mov, dtype=np.float16),
    }


def kernel(prediction, ground_truth, _trace=False, _trace_kwargs=None):
    prediction = np.asarray(prediction, dtype=np.float32)
    ground_truth = np.asarray(ground_truth, dtype=np.float32)
    assert prediction.shape == (B, N, D)
    assert ground_truth.shape == (B, M, D)

    nc = _get_nc()
    in_maps = []
    for c in range(N_CORES):
        b, h = c // 2, c % 2
        x = prediction[b, h * N_SH:(h + 1) * N_SH]
        in_maps.append(_prep_core_inputs(x, ground_truth[b]))

    kw = {}
    if _trace:
        kw = {"trace": True, "trace_cores": [0]}
        if _trace_kwargs:
            kw.update(_trace_kwargs)
    res = run_bass_kernel_spmd(nc, in_maps, list(range(N_CORES)), **kw)

    out = np.zeros(B, dtype=np.float64)
    for c in range(N_CORES):
        out[c // 2] += res.results[c]["out"].astype(np.float64).sum()
    out = (out / N).astype(np.float32)
    if _trace:
        kernel.last_result = res
    return out


# revision 21
# speedup vs baseline: 1.4752x; 1.4752x over previous
"""Half-Chamfer distance kernel for Trainium2 (8 NeuronCores).

Problem: prediction [4, 8192, 3], ground_truth [4, 8192, 3] (f32).
out[b] = mean_n min_m ||pred[b,n] - gt[b,m]||^2

Sharding: core c -> (batch b = c//2, N-half h = c%2). Each core computes
min over all M=8192 gt points for its 4096 prediction points, row-sums;
host combines the per-core [128] partial sums.

Device algorithm (per core), engine-balanced for this HW where the PE
streams moving data at ~1.2 GHz regardless of dtype and PSUM can only be
drained by VectorE (1 elem/cycle via its single PSUM port) and ScalarE
(copy at ~1.09 ns/elem):

  d2[n,m] exactly from fp16-quantized points via K=7 fp16 matmuls:
    stationary rows [x0, x1, x2, 1, 1, x2h, x2l]
    moving rows     [-2y0, -2y1, -2y2, qh, ql, 1, 1]
  (x2h+x2l = |x^|^2, qh+ql = |y^|^2 hi/lo fp16 splits of the f64 norms of
  the QUANTIZED points, so PSUM = |x^-y^|^2 to ~1e-5. Keeping d2 >= 0 in
  the matmul matters: small mins stay accurate in bf16 downstream.)

  Per n-tile (128 preds), M=8192 arrives as 8 PSUM chunks [128,1024]
  (2 matmuls each). Drain split tuned to measured rates:
    - chunks 0,4: consumed directly by VectorE TT-min against a copied
      chunk (1x, 1224ns) -> bf16
    - chunks 1,2,3,5,6,7: ScalarE-copied PSUM->SBUF as bf16 (1114ns)
    - bf16 merge tree on VectorE at 2x (692ns per [128,1024] TT-min)
    - final tensor_reduce min [128,1024] -> dx column (1219ns)
  DVE ~7.1us/n-tile, ACT ~6.7us, PE ~6.8us single-stream -> PE is row-
  tiled 2x (tile_position (0,0)/(32,0), inputs replicated at partition
  offset 32) so two n-tiles' matmuls stream concurrently (~3.4us each).

Tail: relu-clamp + row-sum on device; host sums 128 partials per core.
"""

import numpy as np

import concourse.bass as bass
import concourse.mybir as mybir
from concourse.bass_utils import run_bass_kernel_spmd
from concourse.tile import TileContext

B = 4
N = 8192
M = 8192
D = 3
N_CORES = 8
N_SH = N // 2          # 4096 prediction points per core
KR = 7                 # contraction rows
JC = 512               # cols per matmul (1 PSUM bank of fp32)
CP = 1024              # chunk width (2 matmuls per chunk)
NTILES = N_SH // 128   # 32 n-tiles of 128 partitions
CHUNKS = M // CP       # 8 chunks per n-tile
PSUM_DIRECT = (0, 4)   # chunks drained by DVE straight from PSUM
COPIED = (1, 2, 3, 5, 6, 7)

F32 = mybir.dt.float32
F16 = mybir.dt.float16
BF16 = mybir.dt.bfloat16

_CACHED_NC = None


def _build_nc():
    nc = bass.Bass()
    statx_d = nc.declare_dram_parameter("statx", [KR, N_SH], F16, isOutput=False)
    mov_d = nc.declare_dram_parameter("mov", [KR, M], F16, isOutput=False)
    out_d = nc.declare_dram_parameter("out", [128, 1], F32, isOutput=True)

    with TileContext(nc) as tc:
        with (
            tc.tile_pool(name="const", bufs=1) as cpool,
            tc.tile_pool(name="cp1", bufs=12) as copool,
            tc.tile_pool(name="tr", bufs=8) as trpool,
            tc.tile_pool(name="ps1", bufs=4, space="PSUM") as ps1pool,
        ):
            # inputs replicated at partition offsets 0 and 32 (PE row bands)
            statx = cpool.tile([39, N_SH], F16, tag="statx")
            mov = cpool.tile([39, M], F16, tag="mov")
            dx_all = cpool.tile([128, NTILES], F32, tag="dx")
            nc.sync.dma_start(out=statx[0:KR, :], in_=statx_d[:])
            nc.sync.dma_start(out=statx[32:32 + KR, :], in_=statx_d[:])
            nc.sync.dma_start(out=mov[0:KR, :], in_=mov_d[:])
            nc.sync.dma_start(out=mov[32:32 + KR, :], in_=mov_d[:])

            def tt_min(dst, a, b):
                nc.vector.tensor_tensor(
                    out=dst, in0=a, in1=b, op=mybir.AluOpType.min
                )

            def emit_tree(pend):
                """DVE merge tree for a completed tile (software-pipelined:
                emitted mid-next-iteration so its 6 DVE ops fill the queue
                while the next tile's copies are still being produced)."""
                a0, a1, s2, s3, s6, s7, tp = pend
                b0 = trpool.tile([128, CP], BF16, tag="a")
                tt_min(b0[:], s2[:], s3[:])
                b1 = trpool.tile([128, CP], BF16, tag="a")
                tt_min(b1[:], s6[:], s7[:])
                c0 = trpool.tile([128, CP], BF16, tag="a")
                tt_min(c0[:], a0[:], a1[:])
                c1 = trpool.tile([128, CP], BF16, tag="a")
                tt_min(c1[:], b0[:], b1[:])
                dfin = trpool.tile([128, CP], BF16, tag="a")
                tt_min(dfin[:], c0[:], c1[:])
                nc.vector.tensor_reduce(
                    out=dx_all[:, tp:tp + 1], in_=dfin[:],
                    axis=mybir.AxisListType.X, op=mybir.AluOpType.min,
                )

            pend = None
            for t in range(NTILES):
                base = 32 * (t % 2)     # PE row band for this n-tile
                lhs = statx[base:base + KR, t * 128:(t + 1) * 128]

                def mm_chunk(k2):
                    """PSUM chunk [128,1024] = 2 matmuls of 512 cols"""
                    p = ps1pool.tile([128, CP], F32, tag="ps")
                    for j in range(2):
                        sl = slice(k2 * CP + j * JC, k2 * CP + (j + 1) * JC)
                        nc.tensor.matmul(
                            out=p[:, j * JC:(j + 1) * JC],
                            lhsT=lhs, rhs=mov[base:base + KR, sl],
                            start=True, stop=True,
                            tile_position=(base, 0),
                        )
                    return p

                def copy_chunk(p):
                    s = copool.tile([128, CP], BF16, tag="s")
                    nc.scalar.copy(out=s[:], in_=p[:])
                    return s

                # copy-chunks first: every PSUM slot recycle then waits only
                # on a (fast, early) ScalarE copy, never on late DVE ops
                p1 = mm_chunk(1)
                s1 = copy_chunk(p1)
                p2 = mm_chunk(2)
                s2 = copy_chunk(p2)
                p3 = mm_chunk(3)
                s3 = copy_chunk(p3)
                p5 = mm_chunk(5)
                s5 = copy_chunk(p5)

                # previous tile's merge tree fills the DVE queue here
                if pend is not None:
                    emit_tree(pend)

                p6 = mm_chunk(6)
                s6 = copy_chunk(p6)
                p7 = mm_chunk(7)
                s7 = copy_chunk(p7)

                # PSUM-direct chunks last; their DVE TTs close the tile
                p0 = mm_chunk(0)
                a0 = trpool.tile([128, CP], BF16, tag="a")
                tt_min(a0[:], p0[:], s1[:])
                p4 = mm_chunk(4)
                a1 = trpool.tile([128, CP], BF16, tag="a")
                tt_min(a1[:], p4[:], s5[:])

                pend = (a0, a1, s2, s3, s6, s7, t)
            emit_tree(pend)

            # clamp at 0 (matches reference's maximum(d2, 0) before min)
            nc.vector.tensor_scalar_max(
                out=dx_all[:], in0=dx_all[:], scalar1=0.0
            )
            dxsum = cpool.tile([128, 1], F32, tag="dxsum")
            nc.vector.tensor_reduce(
                out=dxsum[:], in_=dx_all[:],
                axis=mybir.AxisListType.X, op=mybir.AluOpType.add,
            )
            nc.sync.dma_start(out=out_d[:], in_=dxsum[:])

    # Populate .instr bytes for InstISA subclasses; this walrus errors
    # "ISA wrong length" on empty payloads.
    mybir.codegen_inst_isa_subclasses(nc)
    _legalize_for_walrus(nc)
    return nc


def _legalize_for_walrus(nc, max_waits=1):
    """This container's walrus encodes at most one sync-wait per
    instruction (fused-LW matmuls, drains, ...) and cannot encode
    EVENT_SEMAPHORE_RANGE_CLEAR at all.  Spill extra waits onto
    standalone NoOps queued just before on the same engine, and drop the
    tail sem range-clear."""
    RANGE_CLEAR_OPCODE = 176
    for f in nc.m.functions:
        for blk in f.blocks:
            out = []
            for inst in blk.instructions:
                if (
                    type(inst).__name__ == "InstISA"
                    and getattr(inst, "isa_opcode", None) == RANGE_CLEAR_OPCODE
                ):
                    continue
                si = inst.sync_info
                if si is not None and len(si.on_wait) > max_waits:
                    waits = list(si.on_wait)
                    for w in waits[:-max_waits]:
                        out.append(mybir.InstNoOp(
                            name=nc.get_next_instruction_name(),
                            engine=inst.engine,
                            sync_info=mybir.SyncInfo(
                                on_wait=[w], on_update=[]),
                        ))
                    inst.sync_info = mybir.SyncInfo(
                        on_wait=waits[-max_waits:],
                        on_update=list(si.on_update),
                    )
                out.append(inst)
            blk.instructions = out


def _get_nc():
    global _CACHED_NC
    if _CACHED_NC is None:
        _CACHED_NC = _build_nc()
    return _CACHED_NC


def _prep_core_inputs(x, y):
    """x: [N_SH, 3] f32 pred slice; y: [M, 3] f32 gt batch.

    Quantize points to fp16; compute the squared norms of the QUANTIZED
    points in f64 and hi/lo-split them into fp16 pairs, so the matmul's
    fp32 accumulation reconstructs |x^ - y^|^2 to ~1e-5 absolute."""
    xq = x.astype(np.float16)
    yq = y.astype(np.float16)
    x64 = xq.astype(np.float64)
    y64 = yq.astype(np.float64)

    x2 = (x64 * x64).sum(-1)
    x2h = x2.astype(np.float16)
    x2l = (x2 - x2h.astype(np.float64)).astype(np.float16)

    q = (y64 * y64).sum(-1)
    qh = q.astype(np.float16)
    ql = (q - qh.astype(np.float64)).astype(np.float16)

    ones_n = np.ones(N_SH, np.float16)
    ones_m = np.ones(M, np.float16)
    m2 = (-2.0 * y64).astype(np.float16)  # exact: -2 * fp16 value

    statx = np.stack([xq[:, 0], xq[:, 1], xq[:, 2], ones_n, ones_n, x2h, x2l])
    mov = np.stack([m2[:, 0], m2[:, 1], m2[:, 2], qh, ql, ones_m, ones_m])
    return {
        "statx": np.ascontiguousarray(statx, dtype=np.float16),
        "mov": np.ascontiguousarray(mov, dtype=np.float16),
    }


def kernel(prediction, ground_truth, _trace=False, _trace_kwargs=None):
    prediction = np.asarray(prediction, dtype=np.float32)
    ground_truth = np.asarray(ground_truth, dtype=np.float32)
    assert prediction.shape == (B, N, D)
    assert ground_truth.shape == (B, M, D)

    nc = _get_nc()
    in_maps = []
    for c in range(N_CORES):
        b, h = c // 2, c % 2
        x = prediction[b, h * N_SH:(h + 1) * N_SH]
        in_maps.append(_prep_core_inputs(x, ground_truth[b]))

    kw = {}
    if _trace:
        kw = {"trace": True, "trace_cores": [0]}
        if _trace_kwargs:
            kw.update(_trace_kwargs)
    res = run_bass_kernel_spmd(nc, in_maps, list(range(N_CORES)), **kw)

    out = np.zeros(B, dtype=np.float64)
    for c in range(N_CORES):
        out[c // 2] += res.results[c]["out"].astype(np.float64).sum()
    out = (out / N).astype(np.float32)
    if _trace:
        kernel.last_result = res
    return out


# revision 23
# speedup vs baseline: 1.5242x; 1.0332x over previous
"""Half-Chamfer distance kernel for Trainium2 (8 NeuronCores).

Problem: prediction [4, 8192, 3], ground_truth [4, 8192, 3] (f32).
out[b] = mean_n min_m ||pred[b,n] - gt[b,m]||^2

Sharding: core c -> (batch b = c//2, N-half h = c%2). Each core computes
min over all M=8192 gt points for its 4096 prediction points, row-sums;
host combines the per-core [128] partial sums.

Device algorithm (per core), engine-balanced for this HW where the PE
streams moving data at ~1.2 GHz regardless of dtype and PSUM can only be
drained by VectorE (1 elem/cycle via its single PSUM port) and ScalarE
(copy at ~1.09 ns/elem):

  d2[n,m] exactly from fp16-quantized points via K=7 fp16 matmuls:
    stationary rows [x0, x1, x2, 1, 1, x2h, x2l]
    moving rows     [-2y0, -2y1, -2y2, qh, ql, 1, 1]
  (x2h+x2l = |x^|^2, qh+ql = |y^|^2 hi/lo fp16 splits of the f64 norms of
  the QUANTIZED points, so PSUM = |x^-y^|^2 to ~1e-5. Keeping d2 >= 0 in
  the matmul matters: small mins stay accurate in bf16 downstream.)

  Per n-tile (128 preds), M=8192 arrives as 8 PSUM chunks [128,1024]
  (2 matmuls each). Drain split tuned to measured rates:
    - chunks 0,4: consumed directly by VectorE TT-min against a copied
      chunk (1x, 1224ns) -> bf16
    - chunks 1,2,3,5,6,7: ScalarE-copied PSUM->SBUF as bf16 (1114ns)
    - bf16 merge tree on VectorE at 2x (692ns per [128,1024] TT-min)
    - final tensor_reduce min [128,1024] -> dx column (1219ns)
  DVE ~7.1us/n-tile, ACT ~6.7us, PE ~6.8us single-stream -> PE is row-
  tiled 2x (tile_position (0,0)/(32,0), inputs replicated at partition
  offset 32) so two n-tiles' matmuls stream concurrently (~3.4us each).

Tail: relu-clamp + row-sum on device; host sums 128 partials per core.
"""

import numpy as np

import concourse.bass as bass
import concourse.mybir as mybir
from concourse.bass_utils import run_bass_kernel_spmd
from concourse.tile import TileContext

B = 4
N = 8192
M = 8192
D = 3
N_CORES = 8
N_SH = N // 2          # 4096 prediction points per core
KR = 7                 # contraction rows
JC = 512               # cols per matmul (1 PSUM bank of fp32)
CP = 1024              # chunk width (2 matmuls per chunk)
NTILES = N_SH // 128   # 32 n-tiles of 128 partitions
CHUNKS = M // CP       # 8 chunks per n-tile
PSUM_DIRECT = (0, 4)   # chunks drained by DVE straight from PSUM
COPIED = (1, 2, 3, 5, 6, 7)

F32 = mybir.dt.float32
F16 = mybir.dt.float16
BF16 = mybir.dt.bfloat16

_CACHED_NC = None


def _build_nc():
    nc = bass.Bass()
    statx_d = nc.declare_dram_parameter("statx", [KR, N_SH], F16, isOutput=False)
    mov_d = nc.declare_dram_parameter("mov", [KR, M], F16, isOutput=False)
    out_d = nc.declare_dram_parameter("out", [128, 1], F32, isOutput=True)

    with TileContext(nc) as tc:
        with (
            tc.tile_pool(name="const", bufs=1) as cpool,
            tc.tile_pool(name="cp1", bufs=12) as copool,
            tc.tile_pool(name="tr", bufs=8) as trpool,
            tc.tile_pool(name="ps1", bufs=4, space="PSUM") as ps1pool,
        ):
            # inputs replicated at partition offsets 0 and 32 (PE row bands)
            statx = cpool.tile([39, N_SH], F16, tag="statx")
            mov = cpool.tile([39, M], F16, tag="mov")
            dx_all = cpool.tile([128, NTILES], F32, tag="dx")
            # split input DMAs so the first matmuls start after ~1 slice
            # instead of waiting for the whole 112KB transfer
            for b0 in (0, 32):
                for j in range(4):
                    sl = slice(j * (N_SH // 4), (j + 1) * (N_SH // 4))
                    nc.sync.dma_start(out=statx[b0:b0 + KR, sl],
                                      in_=statx_d[:, sl])
                for j in range(8):
                    sl = slice(j * CP, (j + 1) * CP)
                    nc.sync.dma_start(out=mov[b0:b0 + KR, sl],
                                      in_=mov_d[:, sl])

            def tt_min(dst, a, b):
                nc.vector.tensor_tensor(
                    out=dst, in0=a, in1=b, op=mybir.AluOpType.min
                )

            def emit_tree(pend):
                """DVE merge tree for a completed tile (software-pipelined:
                emitted mid-next-iteration so its 6 DVE ops fill the queue
                while the next tile's copies are still being produced)."""
                a0, a1, s2, s3, s6, s7, tp = pend
                b0 = trpool.tile([128, CP], BF16, tag="a")
                tt_min(b0[:], s2[:], s3[:])
                b1 = trpool.tile([128, CP], BF16, tag="a")
                tt_min(b1[:], s6[:], s7[:])
                c0 = trpool.tile([128, CP], BF16, tag="a")
                tt_min(c0[:], a0[:], a1[:])
                c1 = trpool.tile([128, CP], BF16, tag="a")
                tt_min(c1[:], b0[:], b1[:])
                dfin = trpool.tile([128, CP], BF16, tag="a")
                tt_min(dfin[:], c0[:], c1[:])
                efin = trpool.tile([128, CP // 2], BF16, tag="e")
                tt_min(efin[:], dfin[:, :CP // 2], dfin[:, CP // 2:])
                nc.vector.tensor_reduce(
                    out=dx_all[:, tp:tp + 1], in_=efin[:],
                    axis=mybir.AxisListType.X, op=mybir.AluOpType.min,
                )

            pend = None
            for t in range(NTILES):
                base = 32 * (t % 2)     # PE row band for this n-tile
                lhs = statx[base:base + KR, t * 128:(t + 1) * 128]

                def mm_chunk(k2):
                    """PSUM chunk [128,1024] = 2 matmuls of 512 cols"""
                    p = ps1pool.tile([128, CP], F32, tag="ps")
                    for j in range(2):
                        sl = slice(k2 * CP + j * JC, k2 * CP + (j + 1) * JC)
                        nc.tensor.matmul(
                            out=p[:, j * JC:(j + 1) * JC],
                            lhsT=lhs, rhs=mov[base:base + KR, sl],
                            start=True, stop=True,
                            tile_position=(base, 0),
                        )
                    return p

                def copy_chunk(p):
                    s = copool.tile([128, CP], BF16, tag="s")
                    nc.scalar.copy(out=s[:], in_=p[:])
                    return s

                # copy-chunks first: every PSUM slot recycle then waits only
                # on a (fast, early) ScalarE copy, never on late DVE ops
                p1 = mm_chunk(1)
                s1 = copy_chunk(p1)
                p2 = mm_chunk(2)
                s2 = copy_chunk(p2)
                p3 = mm_chunk(3)
                s3 = copy_chunk(p3)
                p5 = mm_chunk(5)
                s5 = copy_chunk(p5)

                # previous tile's merge tree fills the DVE queue here
                if pend is not None:
                    emit_tree(pend)

                p6 = mm_chunk(6)
                s6 = copy_chunk(p6)
                p7 = mm_chunk(7)
                s7 = copy_chunk(p7)

                # PSUM-direct chunks last; their DVE TTs close the tile
                p0 = mm_chunk(0)
                a0 = trpool.tile([128, CP], BF16, tag="a")
                tt_min(a0[:], p0[:], s1[:])
                p4 = mm_chunk(4)
                a1 = trpool.tile([128, CP], BF16, tag="a")
                tt_min(a1[:], p4[:], s5[:])

                pend = (a0, a1, s2, s3, s6, s7, t)
            emit_tree(pend)

            # clamp at 0 (matches reference's maximum(d2, 0) before min)
            nc.vector.tensor_scalar_max(
                out=dx_all[:], in0=dx_all[:], scalar1=0.0
            )
            dxsum = cpool.tile([128, 1], F32, tag="dxsum")
            nc.vector.tensor_reduce(
                out=dxsum[:], in_=dx_all[:],
                axis=mybir.AxisListType.X, op=mybir.AluOpType.add,
            )
            nc.sync.dma_start(out=out_d[:], in_=dxsum[:])

    # Populate .instr bytes for InstISA subclasses; this walrus errors
    # "ISA wrong length" on empty payloads.
    mybir.codegen_inst_isa_subclasses(nc)
    _legalize_for_walrus(nc)
    return nc


def _legalize_for_walrus(nc, max_waits=1):
    """This container's walrus encodes at most one sync-wait per
    instruction (fused-LW matmuls, drains, ...) and cannot encode
    EVENT_SEMAPHORE_RANGE_CLEAR at all.  Spill extra waits onto
    standalone NoOps queued just before on the same engine, and drop the
    tail sem range-clear."""
    RANGE_CLEAR_OPCODE = 176
    for f in nc.m.functions:
        for blk in f.blocks:
            out = []
            for inst in blk.instructions:
                if (
                    type(inst).__name__ == "InstISA"
                    and getattr(inst, "isa_opcode", None) == RANGE_CLEAR_OPCODE
                ):
                    continue
                si = inst.sync_info
                if si is not None and len(si.on_wait) > max_waits:
                    waits = list(si.on_wait)
                    for w in waits[:-max_waits]:
                        out.append(mybir.InstNoOp(
                            name=nc.get_next_instruction_name(),
                            engine=inst.engine,
                            sync_info=mybir.SyncInfo(
                                on_wait=[w], on_update=[]),
                        ))
                    inst.sync_info = mybir.SyncInfo(
                        on_wait=waits[-max_waits:],
                        on_update=list(si.on_update),
                    )
                out.append(inst)
            blk.instructions = out


def _get_nc():
    global _CACHED_NC
    if _CACHED_NC is None:
        _CACHED_NC = _build_nc()
    return _CACHED_NC


def _prep_core_inputs(x, y):
    """x: [N_SH, 3] f32 pred slice; y: [M, 3] f32 gt batch.

    Quantize points to fp16; compute the squared norms of the QUANTIZED
    points in f64 and hi/lo-split them into fp16 pairs, so the matmul's
    fp32 accumulation reconstructs |x^ - y^|^2 to ~1e-5 absolute."""
    xq = x.astype(np.float16)
    yq = y.astype(np.float16)
    x64 = xq.astype(np.float64)
    y64 = yq.astype(np.float64)

    x2 = (x64 * x64).sum(-1)
    x2h = x2.astype(np.float16)
    x2l = (x2 - x2h.astype(np.float64)).astype(np.float16)

    q = (y64 * y64).sum(-1)
    qh = q.astype(np.float16)
    ql = (q - qh.astype(np.float64)).astype(np.float16)

    ones_n = np.ones(N_SH, np.float16)
    ones_m = np.ones(M, np.float16)
    m2 = (-2.0 * y64).astype(np.float16)  # exact: -2 * fp16 value

    statx = np.stack([xq[:, 0], xq[:, 1], xq[:, 2], ones_n, ones_n, x2h, x2l])
    mov = np.stack([m2[:, 0], m2[:, 1], m2[:, 2], qh, ql, ones_m, ones_m])
    return {
        "statx": np.ascontiguousarray(statx, dtype=np.float16),
        "mov": np.ascontiguousarray(mov, dtype=np.float16),
    }


def kernel(prediction, ground_truth, _trace=False, _trace_kwargs=None):
    prediction = np.asarray(prediction, dtype=np.float32)
    ground_truth = np.asarray(ground_truth, dtype=np.float32)
    assert prediction.shape == (B, N, D)
    assert ground_truth.shape == (B, M, D)

    nc = _get_nc()
    in_maps = []
    for c in range(N_CORES):
        b, h = c // 2, c % 2
        x = prediction[b, h * N_SH:(h + 1) * N_SH]
        in_maps.append(_prep_core_inputs(x, ground_truth[b]))

    kw = {}
    if _trace:
        kw = {"trace": True, "trace_cores": [0]}
        if _trace_kwargs:
            kw.update(_trace_kwargs)
    res = run_bass_kernel_spmd(nc, in_maps, list(range(N_CORES)), **kw)

    out = np.zeros(B, dtype=np.float64)
    for c in range(N_CORES):
        out[c // 2] += res.results[c]["out"].astype(np.float64).sum()
    out = (out / N).astype(np.float32)
    if _trace:
        kernel.last_result = res
    return out


# revision 24
# speedup vs baseline: 1.5536x; 1.0192x over previous
"""Half-Chamfer distance kernel for Trainium2 (8 NeuronCores).

Problem: prediction [4, 8192, 3], ground_truth [4, 8192, 3] (f32).
out[b] = mean_n min_m ||pred[b,n] - gt[b,m]||^2

Sharding: core c -> (batch b = c//2, N-half h = c%2). Each core computes
min over all M=8192 gt points for its 4096 prediction points, row-sums;
host combines the per-core [128] partial sums.

Device algorithm (per core), engine-balanced for this HW where the PE
streams moving data at ~1.2 GHz regardless of dtype and PSUM can only be
drained by VectorE (1 elem/cycle via its single PSUM port) and ScalarE
(copy at ~1.09 ns/elem):

  d2[n,m] exactly from fp16-quantized points via K=7 fp16 matmuls:
    stationary rows [x0, x1, x2, 1, 1, x2h, x2l]
    moving rows     [-2y0, -2y1, -2y2, qh, ql, 1, 1]
  (x2h+x2l = |x^|^2, qh+ql = |y^|^2 hi/lo fp16 splits of the f64 norms of
  the QUANTIZED points, so PSUM = |x^-y^|^2 to ~1e-5. Keeping d2 >= 0 in
  the matmul matters: small mins stay accurate in bf16 downstream.)

  Per n-tile (128 preds), M=8192 arrives as 8 PSUM chunks [128,1024]
  (2 matmuls each). Drain split tuned to measured rates:
    - chunks 0,4: consumed directly by VectorE TT-min against a copied
      chunk (1x, 1224ns) -> bf16
    - chunks 1,2,3,5,6,7: ScalarE-copied PSUM->SBUF as bf16 (1114ns)
    - bf16 merge tree on VectorE at 2x (692ns per [128,1024] TT-min)
    - final tensor_reduce min [128,1024] -> dx column (1219ns)
  DVE ~7.1us/n-tile, ACT ~6.7us, PE ~6.8us single-stream -> PE is row-
  tiled 2x (tile_position (0,0)/(32,0), inputs replicated at partition
  offset 32) so two n-tiles' matmuls stream concurrently (~3.4us each).

Tail: relu-clamp + row-sum on device; host sums 128 partials per core.
"""

import numpy as np

import concourse.bass as bass
import concourse.mybir as mybir
from concourse.bass_utils import run_bass_kernel_spmd
from concourse.tile import TileContext

B = 4
N = 8192
M = 8192
D = 3
N_CORES = 8
N_SH = N // 2          # 4096 prediction points per core
KR = 7                 # contraction rows
JC = 512               # cols per matmul (1 PSUM bank of fp32)
CP = 1024              # chunk width (2 matmuls per chunk)
NTILES = N_SH // 128   # 32 n-tiles of 128 partitions
CHUNKS = M // CP       # 8 chunks per n-tile
PSUM_DIRECT = (0, 4)   # chunks drained by DVE straight from PSUM
COPIED = (1, 2, 3, 5, 6, 7)

F32 = mybir.dt.float32
F16 = mybir.dt.float16
BF16 = mybir.dt.bfloat16

_CACHED_NC = None


def _build_nc():
    nc = bass.Bass()
    statx_d = nc.declare_dram_parameter("statx", [KR, N_SH], F16, isOutput=False)
    mov_d = nc.declare_dram_parameter("mov", [KR, M], F16, isOutput=False)
    out_d = nc.declare_dram_parameter("out", [128, 1], F32, isOutput=True)

    with TileContext(nc) as tc:
        with (
            tc.tile_pool(name="const", bufs=1) as cpool,
            tc.tile_pool(name="cp1", bufs=12) as copool,
            tc.tile_pool(name="tr", bufs=8) as trpool,
            tc.tile_pool(name="ps1", bufs=4, space="PSUM") as ps1pool,
        ):
            # inputs replicated at partition offsets 0 and 32 (PE row bands)
            statx = cpool.tile([39, N_SH], F16, tag="statx")
            mov = cpool.tile([39, M], F16, tag="mov")
            dx_all = cpool.tile([128, NTILES], F32, tag="dx")
            # split input DMAs so the first matmuls start after ~1 slice
            # instead of waiting for the whole 112KB transfer; slices land
            # in first-use order (chunk order of tile 0, band 0 first)
            sx = N_SH // 4
            nc.sync.dma_start(out=statx[0:KR, 0:sx], in_=statx_d[:, 0:sx])
            for j in (1, 2, 3, 5):
                sl = slice(j * CP, (j + 1) * CP)
                nc.sync.dma_start(out=mov[0:KR, sl], in_=mov_d[:, sl])
            nc.sync.dma_start(out=statx[32:32 + KR, 0:sx],
                              in_=statx_d[:, 0:sx])
            for j in (6, 7, 0, 4):
                sl = slice(j * CP, (j + 1) * CP)
                nc.sync.dma_start(out=mov[0:KR, sl], in_=mov_d[:, sl])
            for j in range(8):
                sl = slice(j * CP, (j + 1) * CP)
                nc.sync.dma_start(out=mov[32:32 + KR, sl], in_=mov_d[:, sl])
            for b0, jr in ((0, (1, 2, 3)), (32, (1, 2, 3))):
                for j in jr:
                    sl = slice(j * sx, (j + 1) * sx)
                    nc.sync.dma_start(out=statx[b0:b0 + KR, sl],
                                      in_=statx_d[:, sl])

            def tt_min(dst, a, b):
                nc.vector.tensor_tensor(
                    out=dst, in0=a, in1=b, op=mybir.AluOpType.min
                )

            def emit_tree(pend):
                """DVE merge tree for a completed tile (software-pipelined:
                emitted mid-next-iteration so its 6 DVE ops fill the queue
                while the next tile's copies are still being produced)."""
                a0, a1, s2, s3, s6, s7, tp = pend
                b0 = trpool.tile([128, CP], BF16, tag="a")
                tt_min(b0[:], s2[:], s3[:])
                b1 = trpool.tile([128, CP], BF16, tag="a")
                tt_min(b1[:], s6[:], s7[:])
                c0 = trpool.tile([128, CP], BF16, tag="a")
                tt_min(c0[:], a0[:], a1[:])
                c1 = trpool.tile([128, CP], BF16, tag="a")
                tt_min(c1[:], b0[:], b1[:])
                dfin = trpool.tile([128, CP], BF16, tag="a")
                tt_min(dfin[:], c0[:], c1[:])
                efin = trpool.tile([128, CP // 2], BF16, tag="e")
                tt_min(efin[:], dfin[:, :CP // 2], dfin[:, CP // 2:])
                nc.vector.tensor_reduce(
                    out=dx_all[:, tp:tp + 1], in_=efin[:],
                    axis=mybir.AxisListType.X, op=mybir.AluOpType.min,
                )

            pend = None
            for t in range(NTILES):
                base = 32 * (t % 2)     # PE row band for this n-tile
                lhs = statx[base:base + KR, t * 128:(t + 1) * 128]

                def mm_chunk(k2):
                    """PSUM chunk [128,1024] = 2 matmuls of 512 cols"""
                    p = ps1pool.tile([128, CP], F32, tag="ps")
                    for j in range(2):
                        sl = slice(k2 * CP + j * JC, k2 * CP + (j + 1) * JC)
                        nc.tensor.matmul(
                            out=p[:, j * JC:(j + 1) * JC],
                            lhsT=lhs, rhs=mov[base:base + KR, sl],
                            start=True, stop=True,
                            tile_position=(base, 0),
                        )
                    return p

                def copy_chunk(p):
                    s = copool.tile([128, CP], BF16, tag="s")
                    nc.scalar.copy(out=s[:], in_=p[:])
                    return s

                # copy-chunks first: every PSUM slot recycle then waits only
                # on a (fast, early) ScalarE copy, never on late DVE ops
                p1 = mm_chunk(1)
                s1 = copy_chunk(p1)
                p2 = mm_chunk(2)
                s2 = copy_chunk(p2)
                p3 = mm_chunk(3)
                s3 = copy_chunk(p3)
                p5 = mm_chunk(5)
                s5 = copy_chunk(p5)

                # previous tile's merge tree fills the DVE queue here
                if pend is not None:
                    emit_tree(pend)

                p6 = mm_chunk(6)
                s6 = copy_chunk(p6)
                p7 = mm_chunk(7)
                s7 = copy_chunk(p7)

                # PSUM-direct chunks last; their DVE TTs close the tile
                p0 = mm_chunk(0)
                a0 = trpool.tile([128, CP], BF16, tag="a")
                tt_min(a0[:], p0[:], s1[:])
                p4 = mm_chunk(4)
                a1 = trpool.tile([128, CP], BF16, tag="a")
                tt_min(a1[:], p4[:], s5[:])

                pend = (a0, a1, s2, s3, s6, s7, t)
            emit_tree(pend)

            # clamp at 0 (matches reference's maximum(d2, 0) before min)
            nc.vector.tensor_scalar_max(
                out=dx_all[:], in0=dx_all[:], scalar1=0.0
            )
            dxsum = cpool.tile([128, 1], F32, tag="dxsum")
            nc.vector.tensor_reduce(
                out=dxsum[:], in_=dx_all[:],
                axis=mybir.AxisListType.X, op=mybir.AluOpType.add,
            )
            nc.sync.dma_start(out=out_d[:], in_=dxsum[:])

    # Populate .instr bytes for InstISA subclasses; this walrus errors
    # "ISA wrong length" on empty payloads.
    mybir.codegen_inst_isa_subclasses(nc)
    _legalize_for_walrus(nc)
    return nc


def _legalize_for_walrus(nc, max_waits=1):
    """This container's walrus encodes at most one sync-wait per
    instruction (fused-LW matmuls, drains, ...) and cannot encode
    EVENT_SEMAPHORE_RANGE_CLEAR at all.  Spill extra waits onto
    standalone NoOps queued just before on the same engine, and drop the
    tail sem range-clear."""
    RANGE_CLEAR_OPCODE = 176
    for f in nc.m.functions:
        for blk in f.blocks:
            out = []
            for inst in blk.instructions:
                if (
                    type(inst).__name__ == "InstISA"
                    and getattr(inst, "isa_opcode", None) == RANGE_CLEAR_OPCODE
                ):
                    continue
                si = inst.sync_info
                if si is not None and len(si.on_wait) > max_waits:
                    waits = list(si.on_wait)
                    for w in waits[:-max_waits]:
                        out.append(mybir.InstNoOp(
                            name=nc.get_next_instruction_name(),
                            engine=inst.engine,
                            sync_info=mybir.SyncInfo(
                                on_wait=[w], on_update=[]),
                        ))
                    inst.sync_info = mybir.SyncInfo(
                        on_wait=waits[-max_waits:],
                        on_update=list(si.on_update),
                    )
                out.append(inst)
            blk.instructions = out


def _get_nc():
    global _CACHED_NC
    if _CACHED_NC is None:
        _CACHED_NC = _build_nc()
    return _CACHED_NC


def _prep_core_inputs(x, y):
    """x: [N_SH, 3] f32 pred slice; y: [M, 3] f32 gt batch.

    Quantize points to fp16; compute the squared norms of the QUANTIZED
    points in f64 and hi/lo-split them into fp16 pairs, so the matmul's
    fp32 accumulation reconstructs |x^ - y^|^2 to ~1e-5 absolute."""
    xq = x.astype(np.float16)
    yq = y.astype(np.float16)
    x64 = xq.astype(np.float64)
    y64 = yq.astype(np.float64)

    x2 = (x64 * x64).sum(-1)
    x2h = x2.astype(np.float16)
    x2l = (x2 - x2h.astype(np.float64)).astype(np.float16)

    q = (y64 * y64).sum(-1)
    qh = q.astype(np.float16)
    ql = (q - qh.astype(np.float64)).astype(np.float16)

    ones_n = np.ones(N_SH, np.float16)
    ones_m = np.ones(M, np.float16)
    m2 = (-2.0 * y64).astype(np.float16)  # exact: -2 * fp16 value

    statx = np.stack([xq[:, 0], xq[:, 1], xq[:, 2], ones_n, ones_n, x2h, x2l])
    mov = np.stack([m2[:, 0], m2[:, 1], m2[:, 2], qh, ql, ones_m, ones_m])
    return {
        "statx": np.ascontiguousarray(statx, dtype=np.float16),
        "mov": np.ascontiguousarray(mov, dtype=np.float16),
    }


def kernel(prediction, ground_truth, _trace=False, _trace_kwargs=None):
    prediction = np.asarray(prediction, dtype=np.float32)
    ground_truth = np.asarray(ground_truth, dtype=np.float32)
    assert prediction.shape == (B, N, D)
    assert ground_truth.shape == (B, M, D)

    nc = _get_nc()
    in_maps = []
    for c in range(N_CORES):
        b, h = c // 2, c % 2
        x = prediction[b, h * N_SH:(h + 1) * N_SH]
        in_maps.append(_prep_core_inputs(x, ground_truth[b]))

    kw = {}
    if _trace:
        kw = {"trace": True, "trace_cores": [0]}
        if _trace_kwargs:
            kw.update(_trace_kwargs)
    res = run_bass_kernel_spmd(nc, in_maps, list(range(N_CORES)), **kw)

    out = np.zeros(B, dtype=np.float64)
    for c in range(N_CORES):
        out[c // 2] += res.results[c]["out"].astype(np.float64).sum()
    out = (out / N).astype(np.float32)
    if _trace:
        kernel.last_result = res
    return out
